# revision 46
# baseline (speedup 1.0000x reference)
"""CCA (criss-cross attention) on 8 trn2 NeuronCores via Bass/Tile.

Sharding: 8 shards = (batch b in 0..3) x (row-half in 0..1). The half=1
shard vertically FLIPS x/attention_map (and the conv kh taps) on host, so
every core runs the IDENTICAL program computing query rows 0..63 of the
conv grid and output rows 0..127; the host flips the returned half back.

v2: batched DMAs everywhere (8-channel x loads, 16-channel weight loads,
4-channel stage writes, 8-channel residual loads/stores); conv banded
weights halved ([C,128,4,64], one stationary shared by both row-halves,
one N=254 matmul per (channel,kw)); attention-map resize interleaved with
the conv to overlap PE under the conv DMA stream; v_w bounce batched;
phase-7 residual uses bf16 staging with SWDGE cast loads/stores.
"""
import sys
import types
import numpy as np

B, C, H, W = 4, 256, 256, 256
Ca, Cq = 64, 32
h = w = 127
PIX = h * w
NI = 64          # query rows per core
NO = 128         # output rows per core
ESH_E = 45.0     # exp shift for q.k energies
ESH_A = 64.0     # exp shift for affinity energies

_f32 = np.float32


# ---------------------------------------------------------------- host prep
def _resize_mat(n_in, n_out):
    m = np.zeros((n_in, n_out), _f32)
    xs = np.linspace(0.0, n_in - 1.0, n_out)
    x0 = np.floor(xs).astype(np.int64)
    x1 = np.minimum(x0 + 1, n_in - 1)
    wx = (xs - x0).astype(_f32)
    np.add.at(m, (x0, np.arange(n_out)), 1 - wx)
    np.add.at(m, (x1, np.arange(n_out)), wx)
    return m


def _conv_weights(wd):
    # wd: [C,1,4,4] (kh already flipped for half=1 shards)
    # banded weights [128(r), C, 4(kw), 64(m)]: out row ii uses x rows
    # 2*ii+kh of the 128-row half; shared by both halves (rows 0..127 ->
    # conv rows 0..62, rows 128..255 -> conv rows 64..126); col 63 zero.
    # r-major so 16-channel SBUF loads are contiguous per partition.
    out = np.zeros((128, C, 4, 64), _f32)
    for kw in range(4):
        for kh in range(4):
            for ii in range(63):
                out[2 * ii + kh, :, kw, ii] = wd[:, 0, kh, kw]
    return out


# ------------------------------------------------------------- infra fixes
def _install_hookfix():
    try:
        import antenv
        if getattr(antenv, "axon_hooks", None) is not None:
            return
    except ImportError:
        return
    state = {"h": None}
    mod = types.ModuleType("antenv.axon_hooks")
    mod.set_axon_ntff_profile_hook = lambda hk: state.__setitem__("h", hk)
    mod.get_axon_ntff_profile_hook = lambda: state["h"]
    sys.modules["antenv.axon_hooks"] = mod
    antenv.axon_hooks = mod
    try:
        from trn_agent_boot.trn_boot import _ntff_profile_via_ctypes
        hk = _ntff_profile_via_ctypes("/opt/axon/libaxon_pjrt.so")
        if hk is not None:
            mod.set_axon_ntff_profile_hook(hk)
    except Exception:
        pass


def _split_multi_waits(m, mybir):
    # this walrus build supports one sync wait per instruction; split the rest
    ctr = 0
    for f in m.functions:
        for bb in f.blocks:
            out, changed = [], False
            for inst in bb.instructions:
                si = inst.sync_info
                waits = list(si.on_wait) if (si and si.on_wait) else []
                if len(waits) > 1:
                    for wcond in waits[:-1]:
                        ctr += 1
                        nop = mybir.InstNoOp(name=f"waitsplit-{ctr}", ins=[], outs=[])
                        nop.engine = inst.engine
                        nop.sync_info = mybir.SyncInfo(on_wait=[wcond], on_update=[])
                        out.append(nop)
                    inst.sync_info = mybir.SyncInfo(
                        on_wait=[waits[-1]],
                        on_update=list(si.on_update) if si.on_update else [])
                    changed = True
                out.append(inst)
            if changed:
                bb.instructions = out
    return ctr


# ------------------------------------------------------------ device kernel
def _build_kernel():
    import concourse.bass as bass
    import concourse.mybir as mybir
    import concourse.tile as tile
    from concourse.masks import make_identity

    f32 = mybir.dt.float32
    bf16 = mybir.dt.bfloat16
    Exp = mybir.ActivationFunctionType.Exp
    Ident = mybir.ActivationFunctionType.Identity
    Add = mybir.AluOpType.add

    nc = bass.Bass(num_swdge_queues=4)
    xc = nc.dram_tensor("xc", [C, H, W], bf16, kind="ExternalInput")
    amc = nc.dram_tensor("amc", [Ca, H, W], bf16, kind="ExternalInput")
    wconv = nc.dram_tensor("wconv", [128, C, 4, 64], bf16, kind="ExternalInput")
    wdev = nc.dram_tensor("wdev", [C, 16], f32, kind="ExternalInput")
    wqkt = nc.dram_tensor("wqkt", [C, 64], f32, kind="ExternalInput")
    bqk = nc.dram_tensor("bqk", [64, 1], f32, kind="ExternalInput")
    wvt = nc.dram_tensor("wvt", [C, C], f32, kind="ExternalInput")
    bv = nc.dram_tensor("bv", [1, C], f32, kind="ExternalInput")
    rdown = nc.dram_tensor("rdown", [H, h], f32, kind="ExternalInput")
    cdown = nc.dram_tensor("cdown", [W, h], f32, kind="ExternalInput")
    cup = nc.dram_tensor("cup", [h, W], f32, kind="ExternalInput")
    rupg = nc.dram_tensor("rupg", [NI, NO], f32, kind="ExternalInput")
    dmask = nc.dram_tensor("dmask", [NI, h], f32, kind="ExternalInput")
    out_t = nc.dram_tensor("out", [C, NO, W], f32, kind="ExternalOutput")

    with tile.TileContext(nc) as tc:
        cm_consts = tc.tile_pool(name="consts", bufs=1)
        consts = cm_consts.__enter__()
        wqkt_sb = consts.tile([128, 2, 64], bf16)
        nc.gpsimd.dma_start(out=wqkt_sb[:, 0, :], in_=wqkt[0:128, :])
        nc.gpsimd.dma_start(out=wqkt_sb[:, 1, :], in_=wqkt[128:256, :])
        wvt_sb = consts.tile([128, 2, C], bf16)
        nc.gpsimd.dma_start(out=wvt_sb[:, 0, :], in_=wvt[0:128, :])
        nc.gpsimd.dma_start(out=wvt_sb[:, 1, :], in_=wvt[128:256, :])
        bqk_sb = consts.tile([64, 1], f32)
        nc.sync.dma_start(out=bqk_sb, in_=bqk[:, :])
        rdown_sb = consts.tile([128, 2, h], bf16)
        nc.gpsimd.dma_start(out=rdown_sb[:, 0, :], in_=rdown[0:128, :])
        nc.gpsimd.dma_start(out=rdown_sb[:, 1, :], in_=rdown[128:256, :])
        cdown_sb = consts.tile([128, 2, h], bf16)
        nc.gpsimd.dma_start(out=cdown_sb[:, 0, :], in_=cdown[0:128, :])
        nc.gpsimd.dma_start(out=cdown_sb[:, 1, :], in_=cdown[128:256, :])
        cup_sb = consts.tile([h, W], bf16)
        nc.gpsimd.dma_start(out=cup_sb, in_=cup[:, :])
        rupg_sb = consts.tile([NI, NO], bf16)
        nc.gpsimd.dma_start(out=rupg_sb, in_=rupg[:, :])
        dmask8 = consts.tile([128, 4, h], bf16)
        for _d in range(4):
            nc.gpsimd.dma_start(out=dmask8[0:64, _d, :], in_=dmask[:, :])
            nc.gpsimd.dma_start(out=dmask8[64:128, _d, :], in_=dmask[:, :])
        wdev_sb = consts.tile([128, 2, 16], f32)
        nc.sync.dma_start(out=wdev_sb[:, 0, :], in_=wdev[0:128, :])
        nc.sync.dma_start(out=wdev_sb[:, 1, :], in_=wdev[128:256, :])
        ident = consts.tile([128, 128], bf16)
        make_identity(nc, ident)
        eshE_sb = consts.tile([128, 1], f32)
        nc.vector.memset(eshE_sb, -ESH_E)
        eshA_sb = consts.tile([128, 1], f32)
        nc.vector.memset(eshA_sb, -ESH_A)
        # bv broadcast [h, C]
        ones_col = consts.tile([1, 128], bf16)
        nc.vector.memset(ones_col, 1.0)
        bv_row16 = consts.tile([1, C], bf16)
        nc.gpsimd.dma_start(out=bv_row16, in_=bv[:, :])
        bv_bc = consts.tile([128, C], bf16)
        with tc.tile_pool(name="ps_bc", bufs=1, space="PSUM") as ps_bc:
            bvp = ps_bc.tile([128, C], f32)
            nc.tensor.matmul(bvp, ones_col, bv_row16, start=True, stop=True)
            nc.scalar.copy(bv_bc, bvp)

        cm_dram = tc.tile_pool(name="dram", bufs=1, space="DRAM")
        dpool = cm_dram.__enter__()
        stage = dpool.tile([C, PIX], bf16)     # xd bounce buffer in HBM
        vt_dram = dpool.tile([h, h, C], bf16)  # v (column layout) bounce
        a_dram = dpool.tile([Ca, PIX], bf16)   # resized attention map bounce

        # ============ Phase 1: depthwise conv + a-map row-resize ==========
        CG = 8
        cm_at = tc.tile_pool(name="p_at", bufs=1)
        p_at = cm_at.__enter__()
        with tc.tile_pool(name="conv", bufs=4) as cpool, \
             tc.tile_pool(name="convw", bufs=3) as wpool, \
             tc.tile_pool(name="convf", bufs=4) as fpool, \
             tc.tile_pool(name="convs", bufs=1) as s63pool, \
             tc.tile_pool(name="apool", bufs=3) as apool, \
             tc.tile_pool(name="ps_cv", bufs=4, space="PSUM") as ps_cv, \
             tc.tile_pool(name="ps_a", bufs=2, space="PSUM") as ps_a:
            # straddle conv row i=63 on DVE: x rows 126..129, all channels
            x63 = s63pool.tile([128, 2, 4, W], bf16)
            nc.gpsimd.dma_start(out=x63[:, 0, :, :], in_=xc[0:128, 126:130, :])
            nc.gpsimd.dma_start(out=x63[:, 1, :, :], in_=xc[128:256, 126:130, :])
            for ch in range(2):
                acc = fpool.tile([128, h], f32, tag="acc63")
                tmp = fpool.tile([128, h], f32, tag="tmp63")
                for t in range(16):
                    kh, kw = t // 4, t % 4
                    src = x63[:, ch, kh, kw:kw + 2 * (w - 1) + 1:2]
                    if t == 0:
                        nc.vector.tensor_scalar_mul(acc, src, wdev_sb[:, ch, t:t + 1])
                    else:
                        nc.vector.tensor_scalar_mul(tmp, src, wdev_sb[:, ch, t:t + 1])
                        nc.vector.tensor_add(acc, acc, tmp)
                nc.gpsimd.dma_start(
                    out=stage[ch * 128:(ch + 1) * 128, 63 * w:64 * w], in_=acc)

            a_t = p_at.tile([128, 2, Ca, h], bf16)   # [W-half, half, ca, i]
            for g in range(C // CG):                 # 32 groups of 8 channels
                c0 = g * CG
                xt = cpool.tile([128, CG, 2, W], bf16, tag="xt")
                nc.gpsimd.dma_start(
                    out=xt[:, :, 0, :],
                    in_=xc[c0:c0 + CG, 0:128, :].rearrange("c y x -> y c x"))
                nc.gpsimd.dma_start(
                    out=xt[:, :, 1, :],
                    in_=xc[c0:c0 + CG, 128:256, :].rearrange("c y x -> y c x"))
                if g % 2 == 0:
                    wt = wpool.tile([128, 16, 4, 64], bf16, tag="wt")
                    for wh in range(2):
                        nc.gpsimd.dma_start(
                            out=wt[:, 8 * wh:8 * wh + 8].rearrange(
                                "r c k m -> r (c k m)"),
                            in_=wconv[:, c0 + 8 * wh:c0 + 8 * wh + 8].rearrange(
                                "r c k m -> r (c k m)"))
                fU = fpool.tile([63, CG, w], bf16, tag="fU")
                fL = fpool.tile([63, CG, w], bf16, tag="fL")
                for cpair in range(CG // 2):   # 2 channels per PSUM (col-tiled)
                    ci0 = cpair * 2
                    ps = ps_cv.tile([128, 2, 127], f32, tag="cvps")
                    for kw in range(4):
                        for d in range(2):
                            nc.tensor.matmul(
                                ps[64 * d:64 * d + 64],
                                wt[:, (c0 + ci0 + d) % 16, kw, :],
                                xt[:, ci0 + d, :, kw:kw + 2 * (w - 1) + 1:2],
                                start=(kw == 0), stop=(kw == 3))
                    for d in range(2):
                        b = 64 * d
                        if (cpair + d) % 2 == 0:
                            nc.scalar.copy(fU[:, ci0 + d, :], ps[b:b + 63, 0, :])
                            nc.vector.tensor_copy(fL[:, ci0 + d, :], ps[b:b + 63, 1, :])
                        else:
                            nc.vector.tensor_copy(fU[:, ci0 + d, :], ps[b:b + 63, 0, :])
                            nc.scalar.copy(fL[:, ci0 + d, :], ps[b:b + 63, 1, :])
                nc.gpsimd.dma_start(
                    out=stage[c0:c0 + CG, 0:63 * w].rearrange(
                        "c (i j) -> i c j", i=63), in_=fU)
                nc.gpsimd.dma_start(
                    out=stage[c0:c0 + CG, 64 * w:].rearrange(
                        "c (i j) -> i c j", i=63), in_=fL)

                # interleave 2 attention-map channels per group
                if g % 4 == 0:
                    amt = apool.tile([128, 8, 2, W], bf16, tag="amt")
                    ca0 = (g // 4) * 8
                    nc.gpsimd.dma_start(
                        out=amt[:, :, 0, :],
                        in_=amc[ca0:ca0 + 8, 0:128, :].rearrange("c y x -> y c x"))
                    nc.gpsimd.dma_start(
                        out=amt[:, :, 1, :],
                        in_=amc[ca0:ca0 + 8, 128:256, :].rearrange("c y x -> y c x"))
                for dca in range(2):
                    ca = g * 2 + dca
                    ai = ca % 8
                    psr = ps_a.tile([h, W], f32, tag="aps")
                    nc.tensor.matmul(psr, rdown_sb[:, 0, :], amt[:, ai, 0, :],
                                     start=True, stop=False)
                    nc.tensor.matmul(psr, rdown_sb[:, 1, :], amt[:, ai, 1, :],
                                     start=False, stop=True)
                    mid = apool.tile([h, W], bf16, tag="mid")
                    nc.scalar.copy(mid, psr)
                    for wh in range(2):
                        pst = ps_a.tile([128, h], bf16, tag="apst")
                        nc.tensor.transpose(pst, mid[:, wh * 128:(wh + 1) * 128],
                                            ident[0:h, 0:h])
                        nc.vector.tensor_copy(a_t[:, wh, ca, :], pst)

        # a-map col-resize -> a_dram (bounced; reloaded at phase 5)
        with tc.tile_pool(name="ps_a2", bufs=3, space="PSUM") as ps_a2, \
             tc.tile_pool(name="a2tmp", bufs=2) as a2tmp:
            for i0 in range(0, h, 8):
                ib = min(8, h - i0)
                at8 = a2tmp.tile([Ca, 8, h], bf16, tag="at8")
                for di in range(ib):
                    i = i0 + di
                    psc = ps_a2.tile([Ca, h], f32, tag="aps2")
                    nc.tensor.matmul(psc, a_t[:, 0, :, i], cdown_sb[:, 0, :],
                                     start=True, stop=False)
                    nc.tensor.matmul(psc, a_t[:, 1, :, i], cdown_sb[:, 1, :],
                                     start=False, stop=True)
                    nc.scalar.copy(at8[:, di, :], psc)
                nc.gpsimd.dma_start(
                    out=a_dram[:, i0 * w:(i0 + ib) * w],
                    in_=at8[:, 0:ib, :].rearrange("c i j -> c (i j)"))
        cm_at.__exit__(None, None, None)

        # ================ Phase 2+3: q/k and v projections ================
        cm_vw = tc.tile_pool(name="p_vw", bufs=1, side="right")
        p_vw = cm_vw.__enter__()
        v_w = p_vw.tile([h, NI, C], bf16)  # [m, (i, c)] -- stays resident
        cm_xd = tc.tile_pool(name="p_xd", bufs=1, side="right")
        p_xd = cm_xd.__enter__()
        xd0 = p_xd.tile([128, PIX], bf16)
        xd1 = p_xd.tile([128, PIX], bf16)
        for lo in range(0, PIX, 2048):
            hi = min(lo + 2048, PIX)
            nc.gpsimd.dma_start(out=xd0[:, lo:hi], in_=stage[0:128, lo:hi])
            nc.gpsimd.dma_start(out=xd1[:, lo:hi], in_=stage[128:256, lo:hi])

        cm_qk = tc.tile_pool(name="p_qk", bufs=1)
        p_qk = cm_qk.__enter__()
        q_sb = p_qk.tile([32, NI * w], bf16)
        k_sb = p_qk.tile([32, PIX], bf16)
        with tc.tile_pool(name="ps_qk", bufs=3, space="PSUM") as ps_qk:
            CH = 508
            for k in range((PIX + CH - 1) // CH):
                lo = k * CH
                hi = min(lo + CH, PIX)
                ps = ps_qk.tile([64, CH], f32, tag="qkps")
                nc.tensor.matmul(ps[:, 0:hi - lo], wqkt_sb[:, 0, :], xd0[:, lo:hi],
                                 start=True, stop=False)
                nc.tensor.matmul(ps[:, 0:hi - lo], wqkt_sb[:, 1, :], xd1[:, lo:hi],
                                 start=False, stop=True)
                if lo < NI * w:
                    qhi = min(hi, NI * w)
                    nc.scalar.activation(out=q_sb[:, lo:qhi], in_=ps[0:32, 0:qhi - lo],
                                         func=Ident, bias=bqk_sb[0:32], scale=1.0)
                nc.scalar.activation(out=k_sb[:, lo:hi], in_=ps[32:64, 0:hi - lo],
                                     func=Ident, bias=bqk_sb[32:64], scale=1.0)

        with tc.tile_pool(name="ps_v", bufs=3, space="PSUM") as ps_v, \
             tc.tile_pool(name="vttmp", bufs=4) as vttmp:
            for i in range(NI):
                ps = ps_v.tile([h, C], f32, tag="vps")
                nc.tensor.matmul(ps, xd0[:, i * w:(i + 1) * w], wvt_sb[:, 0, :],
                                 start=True, stop=False)
                nc.tensor.matmul(ps, xd1[:, i * w:(i + 1) * w], wvt_sb[:, 1, :],
                                 start=False, stop=True)
                nc.vector.tensor_add(v_w[:, i, :], ps, bv_bc[0:h, :])
            for j0 in range(0, h, 16):
                jb = min(16, h - j0)
                vtt = vttmp.tile([h, 16, C], bf16, tag="vtt")
                for dj in range(jb):
                    j = j0 + dj
                    ps = ps_v.tile([h, C], f32, tag="vps")
                    nc.tensor.matmul(ps, xd0[:, j:j + (h - 1) * w + 1:w],
                                     wvt_sb[:, 0, :], start=True, stop=False)
                    nc.tensor.matmul(ps, xd1[:, j:j + (h - 1) * w + 1:w],
                                     wvt_sb[:, 1, :], start=False, stop=True)
                    nc.vector.tensor_add(vtt[:, dj, :], ps, bv_bc[0:h, :])
                for wh in range(0, jb, 8):
                    we = min(wh + 8, jb)
                    nc.gpsimd.dma_start(
                        out=vt_dram[:, j0 + wh:j0 + we, :].rearrange(
                            "l j c -> l (j c)"),
                        in_=vtt[:, wh:we, :].rearrange("l j c -> l (j c)"))
        cm_xd.__exit__(None, None, None)

        cm_a = tc.tile_pool(name="p_a", bufs=1)
        p_a = cm_a.__enter__()
        a_sb = p_a.tile([Ca, PIX], bf16)
        nc.sync.dma_start(out=a_sb, in_=a_dram)

        # ================= Phase 5: energies + first softmax ==============
        cm_e = tc.tile_pool(name="p_e", bufs=1, side="right")
        p_e = cm_e.__enter__()
        peH2 = p_e.tile([128, 16, 4, h], bf16)  # [(jhalf,i), blk, dj, l]
        peW = p_e.tile([h, NI, h], bf16)   # [j, (i, m)]
        zeH2 = p_e.tile([128, 64], f32)
        zaH2 = p_e.tile([128, 64], f32)
        zeW = p_e.tile([h, NI], f32)
        zaW = p_e.tile([h, NI], f32)
        with tc.tile_pool(name="espool", bufs=3) as espool, \
             tc.tile_pool(name="ps_e", bufs=3, space="PSUM") as ps_e:
            Red = mybir.AluOpType.add
            AxX = mybir.AxisListType.X
            for blk in range(16):
                jj0 = blk * 4
                pe = ps_e.tile([128, 4, 128], f32, tag="pe")
                pa = ps_e.tile([128, 4, 128], f32, tag="pa")
                for jhalf in range(2):
                    b = 64 * jhalf
                    for dj in range(4 if jhalf == 0 else min(4, 63 - jj0)):
                        jj = jhalf * 64 + jj0 + dj
                        nc.tensor.matmul(pe[b:b + 64, dj, 0:h],
                                         q_sb[:, jj:jj + 63 * w + 1:w],
                                         k_sb[:, jj:jj + (h - 1) * w + 1:w],
                                         start=True, stop=True)
                        nc.tensor.matmul(pa[b:b + 64, dj, 0:h],
                                         a_sb[:, jj:jj + 63 * w + 1:w],
                                         a_sb[:, jj:jj + (h - 1) * w + 1:w],
                                         start=True, stop=True)
                nc.vector.tensor_add(pe[:, :, 0:h], pe[:, :, 0:h], dmask8)
                nc.vector.tensor_add(pa[:, :, 0:h], pa[:, :, 0:h], dmask8)
                ee = espool.tile([128, 4, h], bf16, tag="ee")
                nc.scalar.activation(out=ee, in_=pe[:, :, 0:h],
                                     func=Exp, bias=eshE_sb)
                ea = espool.tile([128, 4, h], bf16, tag="ea")
                nc.scalar.activation(out=ea, in_=pa[:, :, 0:h],
                                     func=Exp, bias=eshA_sb)
                nc.vector.tensor_reduce(zeH2[:, jj0:jj0 + 4], ee, axis=AxX, op=Red)
                nc.vector.tensor_reduce(zaH2[:, jj0:jj0 + 4], ea, axis=AxX, op=Red)
                nc.vector.tensor_mul(peH2[:, blk, :, :], ee, ea)
        with tc.tile_pool(name="espool2", bufs=3) as espool, \
             tc.tile_pool(name="ps_e2", bufs=3, space="PSUM") as ps_e:
            Red = mybir.AluOpType.add
            AxX = mybir.AxisListType.X
            for i0 in range(0, NI, 4):
                pe = ps_e.tile([h, 4, 128], f32, tag="pew")
                pa = ps_e.tile([h, 4, 128], f32, tag="paw")
                for di in range(4):
                    ii = i0 + di
                    nc.tensor.matmul(pe[:, di, 0:h], q_sb[:, ii * w:(ii + 1) * w],
                                     k_sb[:, ii * w:(ii + 1) * w],
                                     start=True, stop=True)
                    nc.tensor.matmul(pa[:, di, 0:h], a_sb[:, ii * w:(ii + 1) * w],
                                     a_sb[:, ii * w:(ii + 1) * w],
                                     start=True, stop=True)
                ee = espool.tile([h, 4, h], bf16, tag="eew")
                nc.scalar.activation(out=ee, in_=pe[:, :, 0:h], func=Exp, bias=eshE_sb[0:h])
                ea = espool.tile([h, 4, h], bf16, tag="eaw")
                nc.scalar.activation(out=ea, in_=pa[:, :, 0:h], func=Exp, bias=eshA_sb[0:h])
                nc.vector.tensor_reduce(zeW[:, i0:i0 + 4], ee, axis=AxX, op=Red)
                nc.vector.tensor_reduce(zaW[:, i0:i0 + 4], ea, axis=AxX, op=Red)
                nc.vector.tensor_mul(peW[:, i0:i0 + 4, :], ee, ea)
        cm_a.__exit__(None, None, None)
        cm_qk.__exit__(None, None, None)

        # v (column layout) reload -- overlaps 6a (space freed by q/k/a_sb)
        cm_vt = tc.tile_pool(name="p_vt", bufs=1)
        p_vt = cm_vt.__enter__()
        v_t = p_vt.tile([h, h, C], bf16)   # [l, (j, c)]
        for j0 in range(0, h, 16):
            jb = min(16, h - j0)
            nc.gpsimd.dma_start(
                out=v_t[:, j0:j0 + jb, :].rearrange("l j c -> l (j c)"),
                in_=vt_dram[:, j0:j0 + jb, :].rearrange("l j c -> l (j c)"))

        # normalizer merge (stacked): rs = 1/((zeH+zeW^T)*(zaH+zaW^T))
        rs_H2 = consts.tile([128, 64], f32)
        rs_W = consts.tile([h, NI], f32)
        zsH2 = consts.tile([128, 64], f32)
        zsW = consts.tile([h, NI], f32)
        zs_tot = consts.tile([h, NI], f32)
        with tc.tile_pool(name="ps_s", bufs=2, space="PSUM") as ps_s, \
             tc.tile_pool(name="stmp", bufs=1) as stmp:
            zeW16 = stmp.tile([h, NI], bf16)
            zaW16 = stmp.tile([h, NI], bf16)
            nc.vector.tensor_copy(zeW16, zeW)
            nc.vector.tensor_copy(zaW16, zaW)
            pz1 = ps_s.tile([128, NI], bf16, tag="pz")
            nc.tensor.transpose(pz1[0:64, :], zeW16[0:64, :], ident[0:64, 0:64])
            nc.tensor.transpose(pz1[64:128, 0:63], zeW16[64:127, :],
                                ident[64:127, 64:127])
            ze_tot = stmp.tile([128, 64], f32)
            nc.vector.tensor_add(ze_tot, zeH2, pz1[:, 0:64])
            pz2 = ps_s.tile([128, NI], bf16, tag="pz")
            nc.tensor.transpose(pz2[0:64, :], zaW16[0:64, :], ident[0:64, 0:64])
            nc.tensor.transpose(pz2[64:128, 0:63], zaW16[64:127, :],
                                ident[64:127, 64:127])
            za_tot = stmp.tile([128, 64], f32)
            nc.vector.tensor_add(za_tot, zaH2, pz2[:, 0:64])
            nc.vector.tensor_mul(rs_H2, ze_tot, za_tot)
            nc.vector.reciprocal(rs_H2, rs_H2)
            rs16 = stmp.tile([128, 64], bf16)
            nc.vector.tensor_copy(rs16, rs_H2)
            pz3 = ps_s.tile([128, NI], bf16, tag="pzw")
            nc.tensor.transpose(pz3[0:64, :], rs16[0:64, :], ident[0:64, 0:64])
            nc.tensor.transpose(pz3[64:128, :], rs16[64:128, :],
                                ident[64:128, 64:128])
            nc.vector.tensor_copy(rs_W, pz3[0:h, :])

        # ====== Phase 6a: second softmax -> transposed attention weights ===
        cm_es = tc.tile_pool(name="p_es", bufs=1)
        p_es = cm_es.__enter__()
        esT_H2 = p_es.tile([h, 16, 4, 128], bf16)  # [l, blk, dj, (jhalf,i)]
        esT_W = p_es.tile([h, NI, h], bf16)   # [m, (i, j)]
        with tc.tile_pool(name="spool", bufs=2) as spool, \
             tc.tile_pool(name="ps_f", bufs=3, space="PSUM") as ps_f:
            Red = mybir.AluOpType.add
            AxX = mybir.AxisListType.X
            for blk in range(16):
                jj0 = blk * 4
                p1 = spool.tile([128, 4, h], bf16, tag="p1")
                rs_bc = rs_H2[:, jj0:jj0 + 4].to_broadcast((128, 4, h))
                nc.vector.tensor_tensor(out=p1, in0=peH2[:, blk, :, :],
                                        in1=rs_bc, op=mybir.AluOpType.mult)
                es = spool.tile([128, 4, h], bf16, tag="es")
                nc.scalar.activation(out=es, in_=p1, func=Exp)
                nc.vector.tensor_reduce(zsH2[:, jj0:jj0 + 4], es, axis=AxX, op=Red)
                pt = ps_f.tile([h, 4, 128], bf16, tag="pt")
                for dj in range(4):
                    nc.tensor.transpose(pt[:, dj, :], es[:, dj, :], ident)
                nc.vector.tensor_copy(esT_H2[:, blk, :, :], pt)
            for i0 in range(0, NI, 4):
                p1 = spool.tile([h, 4, h], bf16, tag="p1w")
                rs_bc = rs_W[:, i0:i0 + 4].to_broadcast((h, 4, h))
                nc.vector.tensor_tensor(out=p1, in0=peW[:, i0:i0 + 4, :],
                                        in1=rs_bc, op=mybir.AluOpType.mult)
                es = spool.tile([h, 4, h], bf16, tag="esw")
                nc.scalar.activation(out=es, in_=p1, func=Exp)
                nc.vector.tensor_reduce(zsW[:, i0:i0 + 4], es, axis=AxX, op=Red)
                pt = ps_f.tile([h, 8, 128], bf16, tag="ptw")
                for di in range(4):
                    nc.tensor.transpose(pt[:, di, 0:h], es[:, di, :], ident[0:h, 0:h])
                nc.vector.tensor_copy(esT_W[:, i0:i0 + 4, :], pt[:, 0:4, 0:h])
        cm_e.__exit__(None, None, None)

        cm_o = tc.tile_pool(name="p_o", bufs=1)
        p_o = cm_o.__enter__()
        Tbuf = p_o.tile([h, NI, C], bf16)     # [j, (i, c)]

        # zs_tot = 1/(zsH^T + zsW)  [h, NI]
        with tc.tile_pool(name="ps_m", bufs=2, space="PSUM") as ps_m, \
             tc.tile_pool(name="mtmp", bufs=1) as mtmp:
            zsH16 = mtmp.tile([128, 64], bf16)
            nc.vector.tensor_copy(zsH16, zsH2)
            pzs = ps_m.tile([128, NI], bf16, tag="pzs")
            nc.tensor.transpose(pzs[0:64, :], zsH16[0:64, :], ident[0:64, 0:64])
            nc.tensor.transpose(pzs[64:128, :], zsH16[64:128, :],
                                ident[64:128, 64:128])
            nc.vector.tensor_add(zs_tot, zsW, pzs[0:h, :])
            nc.vector.reciprocal(zs_tot, zs_tot)

        # ====== Phase 6b: attention apply (outW then outH) -> Tbuf ========
        with tc.tile_pool(name="ps_f3", bufs=3, space="PSUM") as ps_f3:
            for i0 in range(0, NI, 2):
                po = ps_f3.tile([h, 2, C], f32, tag="pow")
                for di in range(2):
                    nc.tensor.matmul(po[:, di, :], esT_W[:, i0 + di, :],
                                     v_w[:, i0 + di, :], start=True, stop=True)
                nc.vector.tensor_copy(Tbuf[:, i0:i0 + 2, :], po)
        cm_vw.__exit__(None, None, None)
        with tc.tile_pool(name="p_oh", bufs=2) as p_oh, \
             tc.tile_pool(name="ps_f2", bufs=3, space="PSUM") as ps_f2:
            for ch in range(2):
                outHh = p_oh.tile([NI, h, 128], bf16, tag="outHh")
                for jb in range(32):
                    jhalf, blk = jb // 16, jb % 16
                    j0 = jhalf * 64 + blk * 4
                    jn = min(4, h - j0)
                    po = ps_f2.tile([NI, 4, 128], f32, tag="po")
                    for dj in range(jn):
                        nc.tensor.matmul(
                            po[:, dj, :],
                            esT_H2[:, blk, dj, 64 * jhalf:64 * jhalf + 64],
                            v_t[:, j0 + dj, ch * 128:(ch + 1) * 128],
                            start=True, stop=True)
                    if (j0 // 4) % 2 == 0:
                        nc.vector.tensor_copy(outHh[:, j0:j0 + jn, :], po[:, 0:jn, :])
                    else:
                        nc.scalar.copy(outHh[:, j0:j0 + jn, :], po[:, 0:jn, :])
                for cb0 in range(0, 128, 4):
                    ptr = ps_f2.tile([h, 8, 128], bf16, tag="ptr")
                    for db in range(4):
                        nc.tensor.transpose(ptr[:, db, 0:NI], outHh[:, :, cb0 + db],
                                            ident[0:NI, 0:NI])
                    dst = Tbuf[:, :, ch * 128 + cb0:ch * 128 + cb0 + 4]
                    src = ptr[:, 0:4, 0:NI].rearrange("p c i -> p i c")
                    nc.vector.tensor_tensor(out=dst, in0=dst, in1=src, op=Add)

        # T *= zs_tot
        for i in range(NI):
            nc.vector.tensor_scalar_mul(Tbuf[:, i, :], Tbuf[:, i, :],
                                        zs_tot[:, i:i + 1])

        # ====== Phase 7: upsample + residual, pipelined in 64-ch blocks ====
        Rg = 8    # residual channels per DMA group
        CQ = 64   # channels per pipeline block
        with tc.tile_pool(name="p_u", bufs=1) as p_u, \
             tc.tile_pool(name="p_r", bufs=3) as p_r, \
             tc.tile_pool(name="ps_u", bufs=3, space="PSUM") as ps_u, \
             tc.tile_pool(name="ps_r", bufs=3, space="PSUM") as ps_r:
            for cq in range(C // CQ):                # 4 blocks of 64 channels
                cqb = cq * CQ
                U2 = p_u.tile([NI, CQ, W], bf16, tag="U2")   # [i, c, xo]
                for c in range(CQ):
                    psc = ps_u.tile([NI, W], f32, tag="ups")
                    nc.tensor.matmul(psc, Tbuf[:, :, cqb + c], cup_sb,
                                     start=True, stop=True)
                    if c % 2 == 0:
                        nc.scalar.copy(U2[:, c, :], psc)
                    else:
                        nc.vector.tensor_copy(U2[:, c, :], psc)
                U2f = U2.rearrange("i c xo -> i (c xo)")
                for g8 in range(CQ // Rg):           # 8 groups of 8 channels
                    c0 = cqb + g8 * Rg
                    xr8 = p_r.tile([NO, Rg, W], bf16, tag="xr8")
                    nc.gpsimd.dma_start(
                        out=xr8,
                        in_=xc[c0:c0 + Rg, 0:NO, :].rearrange("c y xo -> y c xo"))
                    o8 = p_r.tile([NO, Rg, W], f32, tag="o8")
                    for k in range(Rg // 2):         # 2 channels per matmul
                        kk = g8 * (Rg // 2) + k
                        ps = ps_r.tile([NO, 512], f32, tag="rps")
                        nc.tensor.matmul(ps, rupg_sb,
                                         U2f[:, kk * 512:(kk + 1) * 512],
                                         start=True, stop=True)
                        nc.vector.tensor_tensor(
                            out=o8[:, 2 * k:2 * k + 2, :].rearrange("p c xo -> p (c xo)"),
                            in0=ps,
                            in1=xr8[:, 2 * k:2 * k + 2, :].rearrange("p c xo -> p (c xo)"),
                            op=Add)
                    nc.gpsimd.dma_start(
                        out=out_t[c0:c0 + Rg, :, :].rearrange("c y xo -> y c xo"),
                        in_=o8)
        cm_o.__exit__(None, None, None)
        cm_es.__exit__(None, None, None)
        cm_vt.__exit__(None, None, None)
        cm_dram.__exit__(None, None, None)
        cm_consts.__exit__(None, None, None)

    return nc


_NC_CACHE = {}


def _get_nc():
    if "nc" not in _NC_CACHE:
        _install_hookfix()
        import concourse.mybir as mybir
        nc = _build_kernel()
        _split_multi_waits(nc.m, mybir)
        _NC_CACHE["nc"] = nc
    return _NC_CACHE["nc"]


def _host_inputs(x, attention_map, w_down, wq, bq, wk, bk, wv, bv, gamma):
    import ml_dtypes
    bfl = ml_dtypes.bfloat16
    x = np.ascontiguousarray(x, _f32)
    attention_map = np.ascontiguousarray(attention_map, _f32)
    w_down = np.asarray(w_down, _f32)
    gamma_v = float(np.asarray(gamma).reshape(-1)[0])

    wqkt = np.concatenate([np.asarray(wq, _f32).T, np.asarray(wk, _f32).T], axis=1)
    bqk = np.concatenate([np.asarray(bq, _f32), np.asarray(bk, _f32)])[:, None]
    wvt = np.asarray(wv, _f32).T.copy()
    bv_ = np.asarray(bv, _f32)[None, :]
    rdown = _resize_mat(H, h)
    cdown = _resize_mat(W, h)
    cup = _resize_mat(h, W)
    rupg = np.ascontiguousarray(_resize_mat(h, H)[:NI, :NO] * gamma_v)
    dmask = np.zeros((NI, h), _f32)
    dmask[np.arange(NI), np.arange(NI)] = -30000.0

    wconv_n = _conv_weights(w_down).astype(bfl)
    wconv_f = _conv_weights(w_down[:, :, ::-1, :]).astype(bfl)
    wdev_n = np.ascontiguousarray(w_down[:, 0].reshape(C, 16))
    wdev_f = np.ascontiguousarray(w_down[:, 0, ::-1, :].reshape(C, 16))

    shared = dict(wqkt=wqkt, bqk=bqk, wvt=wvt, bv=bv_, rdown=rdown,
                  cdown=cdown, cup=cup, rupg=rupg, dmask=dmask)
    in_maps = []
    x16 = x.astype(bfl)
    am16 = attention_map.astype(bfl)
    for core in range(8):
        b, half = core // 2, core % 2
        if half == 0:
            m = dict(xc=x16[b], amc=am16[b], wconv=wconv_n, wdev=wdev_n)
        else:
            m = dict(xc=np.ascontiguousarray(x16[b, :, ::-1, :]),
                     amc=np.ascontiguousarray(am16[b, :, ::-1, :]),
                     wconv=wconv_f, wdev=wdev_f)
        m.update(shared)
        in_maps.append(m)
    return in_maps


def kernel(x, attention_map, w_down, wq, bq, wk, bk, wv, bv, gamma):
    _install_hookfix()
    from concourse import bass_utils

    nc = _get_nc()
    in_maps = _host_inputs(x, attention_map, w_down, wq, bq, wk, bk, wv, bv, gamma)
    res = bass_utils.run_bass_kernel_spmd(nc, in_maps, core_ids=list(range(8)))
    out = np.empty((B, C, H, W), _f32)
    for core in range(8):
        b, half = core // 2, core % 2
        o = res.results[core]["out"]
        if half == 0:
            out[b, :, 0:NO, :] = o
        else:
            out[b, :, NO:H, :] = o[:, ::-1, :]
    return out


# revision 47
# speedup vs baseline: 1.0762x; 1.0762x over previous
"""CCA (criss-cross attention) on 8 trn2 NeuronCores via Bass/Tile.

Sharding: 8 shards = (batch b in 0..3) x (row-half in 0..1). The half=1
shard vertically FLIPS x/attention_map (and the conv kh taps) on host, so
every core runs the IDENTICAL program computing query rows 0..63 of the
conv grid and output rows 0..127; the host flips the returned half back.

v2: batched DMAs everywhere (8-channel x loads, 16-channel weight loads,
4-channel stage writes, 8-channel residual loads/stores); conv banded
weights halved ([C,128,4,64], one stationary shared by both row-halves,
one N=254 matmul per (channel,kw)); attention-map resize interleaved with
the conv to overlap PE under the conv DMA stream; v_w bounce batched;
phase-7 residual uses bf16 staging with SWDGE cast loads/stores.
"""
import sys
import types
import numpy as np

B, C, H, W = 4, 256, 256, 256
Ca, Cq = 64, 32
h = w = 127
PIX = h * w
NI = 64          # query rows per core
NO = 128         # output rows per core
ESH_E = 45.0     # exp shift for q.k energies
ESH_A = 64.0     # exp shift for affinity energies

_f32 = np.float32


# ---------------------------------------------------------------- host prep
def _resize_mat(n_in, n_out):
    m = np.zeros((n_in, n_out), _f32)
    xs = np.linspace(0.0, n_in - 1.0, n_out)
    x0 = np.floor(xs).astype(np.int64)
    x1 = np.minimum(x0 + 1, n_in - 1)
    wx = (xs - x0).astype(_f32)
    np.add.at(m, (x0, np.arange(n_out)), 1 - wx)
    np.add.at(m, (x1, np.arange(n_out)), wx)
    return m


def _conv_weights(wd):
    # wd: [C,1,4,4] (kh already flipped for half=1 shards)
    # banded weights [128(r), C, 4(kw), 64(m)]: out row ii uses x rows
    # 2*ii+kh of the 128-row half; shared by both halves (rows 0..127 ->
    # conv rows 0..62, rows 128..255 -> conv rows 64..126); col 63 zero.
    # r-major so 16-channel SBUF loads are contiguous per partition.
    out = np.zeros((128, C, 4, 64), _f32)
    for kw in range(4):
        for kh in range(4):
            for ii in range(63):
                out[2 * ii + kh, :, kw, ii] = wd[:, 0, kh, kw]
    return out


# ------------------------------------------------------------- infra fixes
def _install_hookfix():
    try:
        import antenv
        if getattr(antenv, "axon_hooks", None) is not None:
            return
    except ImportError:
        return
    state = {"h": None}
    mod = types.ModuleType("antenv.axon_hooks")
    mod.set_axon_ntff_profile_hook = lambda hk: state.__setitem__("h", hk)
    mod.get_axon_ntff_profile_hook = lambda: state["h"]
    sys.modules["antenv.axon_hooks"] = mod
    antenv.axon_hooks = mod
    try:
        from trn_agent_boot.trn_boot import _ntff_profile_via_ctypes
        hk = _ntff_profile_via_ctypes("/opt/axon/libaxon_pjrt.so")
        if hk is not None:
            mod.set_axon_ntff_profile_hook(hk)
    except Exception:
        pass


def _split_multi_waits(m, mybir):
    # this walrus build supports one sync wait per instruction; split the rest
    ctr = 0
    for f in m.functions:
        for bb in f.blocks:
            out, changed = [], False
            for inst in bb.instructions:
                si = inst.sync_info
                waits = list(si.on_wait) if (si and si.on_wait) else []
                if len(waits) > 1:
                    for wcond in waits[:-1]:
                        ctr += 1
                        nop = mybir.InstNoOp(name=f"waitsplit-{ctr}", ins=[], outs=[])
                        nop.engine = inst.engine
                        nop.sync_info = mybir.SyncInfo(on_wait=[wcond], on_update=[])
                        out.append(nop)
                    inst.sync_info = mybir.SyncInfo(
                        on_wait=[waits[-1]],
                        on_update=list(si.on_update) if si.on_update else [])
                    changed = True
                out.append(inst)
            if changed:
                bb.instructions = out
    return ctr


# ------------------------------------------------------------ device kernel
def _build_kernel():
    import concourse.bass as bass
    import concourse.mybir as mybir
    import concourse.tile as tile
    from concourse.masks import make_identity

    f32 = mybir.dt.float32
    bf16 = mybir.dt.bfloat16
    Exp = mybir.ActivationFunctionType.Exp
    Ident = mybir.ActivationFunctionType.Identity
    Add = mybir.AluOpType.add

    nc = bass.Bass(num_swdge_queues=4)
    xc = nc.dram_tensor("xc", [C, H, W], bf16, kind="ExternalInput")
    amc = nc.dram_tensor("amc", [Ca, H, W], bf16, kind="ExternalInput")
    wconv = nc.dram_tensor("wconv", [128, C, 4, 64], bf16, kind="ExternalInput")
    wdev = nc.dram_tensor("wdev", [C, 16], f32, kind="ExternalInput")
    wqkt = nc.dram_tensor("wqkt", [C, 64], f32, kind="ExternalInput")
    bqk = nc.dram_tensor("bqk", [64, 1], f32, kind="ExternalInput")
    wvt = nc.dram_tensor("wvt", [C, C], f32, kind="ExternalInput")
    bv = nc.dram_tensor("bv", [1, C], f32, kind="ExternalInput")
    rdown = nc.dram_tensor("rdown", [H, h], f32, kind="ExternalInput")
    cdown = nc.dram_tensor("cdown", [W, h], f32, kind="ExternalInput")
    cup = nc.dram_tensor("cup", [h, W], f32, kind="ExternalInput")
    rupg = nc.dram_tensor("rupg", [NI, NO], f32, kind="ExternalInput")
    dmask = nc.dram_tensor("dmask", [NI, h], f32, kind="ExternalInput")
    out_t = nc.dram_tensor("out", [C, NO, W], f32, kind="ExternalOutput")

    with tile.TileContext(nc) as tc:
        cm_consts = tc.tile_pool(name="consts", bufs=1)
        consts = cm_consts.__enter__()
        wqkt_sb = consts.tile([128, 2, 64], bf16)
        nc.gpsimd.dma_start(out=wqkt_sb[:, 0, :], in_=wqkt[0:128, :])
        nc.gpsimd.dma_start(out=wqkt_sb[:, 1, :], in_=wqkt[128:256, :])
        wvt_sb = consts.tile([128, 2, C], bf16)
        nc.gpsimd.dma_start(out=wvt_sb[:, 0, :], in_=wvt[0:128, :])
        nc.gpsimd.dma_start(out=wvt_sb[:, 1, :], in_=wvt[128:256, :])
        bqk_sb = consts.tile([64, 1], f32)
        nc.sync.dma_start(out=bqk_sb, in_=bqk[:, :])
        rdown_sb = consts.tile([128, 2, h], bf16)
        nc.gpsimd.dma_start(out=rdown_sb[:, 0, :], in_=rdown[0:128, :])
        nc.gpsimd.dma_start(out=rdown_sb[:, 1, :], in_=rdown[128:256, :])
        cdown_sb = consts.tile([128, 2, h], bf16)
        nc.gpsimd.dma_start(out=cdown_sb[:, 0, :], in_=cdown[0:128, :])
        nc.gpsimd.dma_start(out=cdown_sb[:, 1, :], in_=cdown[128:256, :])
        cup_sb = consts.tile([h, W], bf16)
        nc.gpsimd.dma_start(out=cup_sb, in_=cup[:, :])
        rupg_sb = consts.tile([NI, NO], bf16)
        nc.gpsimd.dma_start(out=rupg_sb, in_=rupg[:, :])
        dmask8 = consts.tile([128, 4, h], bf16)
        for _d in range(4):
            nc.gpsimd.dma_start(out=dmask8[0:64, _d, :], in_=dmask[:, :])
            nc.gpsimd.dma_start(out=dmask8[64:128, _d, :], in_=dmask[:, :])
        wdev_sb = consts.tile([128, 2, 16], f32)
        nc.sync.dma_start(out=wdev_sb[:, 0, :], in_=wdev[0:128, :])
        nc.sync.dma_start(out=wdev_sb[:, 1, :], in_=wdev[128:256, :])
        ident = consts.tile([128, 128], bf16)
        make_identity(nc, ident)
        eshE_sb = consts.tile([128, 1], f32)
        nc.vector.memset(eshE_sb, -ESH_E)
        eshA_sb = consts.tile([128, 1], f32)
        nc.vector.memset(eshA_sb, -ESH_A)
        # bv broadcast [h, C]
        ones_col = consts.tile([1, 128], bf16)
        nc.vector.memset(ones_col, 1.0)
        bv_row16 = consts.tile([1, C], bf16)
        nc.gpsimd.dma_start(out=bv_row16, in_=bv[:, :])
        bv_bc = consts.tile([128, C], bf16)
        with tc.tile_pool(name="ps_bc", bufs=1, space="PSUM") as ps_bc:
            bvp = ps_bc.tile([128, C], f32)
            nc.tensor.matmul(bvp, ones_col, bv_row16, start=True, stop=True)
            nc.scalar.copy(bv_bc, bvp)

        cm_dram = tc.tile_pool(name="dram", bufs=1, space="DRAM")
        dpool = cm_dram.__enter__()
        stage = dpool.tile([C, PIX], bf16)     # xd bounce buffer in HBM
        vt_dram = dpool.tile([h, h, C], bf16)  # v (column layout) bounce
        a_dram = dpool.tile([Ca, PIX], bf16)   # resized attention map bounce

        # ============ Phase 1: depthwise conv + a-map row-resize ==========
        CG = 8
        cm_at = tc.tile_pool(name="p_at", bufs=1)
        p_at = cm_at.__enter__()
        with tc.tile_pool(name="conv", bufs=3) as cpool, \
             tc.tile_pool(name="convw", bufs=2) as wpool, \
             tc.tile_pool(name="convf", bufs=4) as fpool, \
             tc.tile_pool(name="convs", bufs=1) as s63pool, \
             tc.tile_pool(name="apool", bufs=2) as apool, \
             tc.tile_pool(name="ps_cv", bufs=4, space="PSUM") as ps_cv, \
             tc.tile_pool(name="ps_a", bufs=2, space="PSUM") as ps_a:
            # straddle conv row i=63 on DVE: x rows 126..129, all channels
            x63 = s63pool.tile([128, 2, 4, W], bf16)
            nc.gpsimd.dma_start(out=x63[:, 0, :, :], in_=xc[0:128, 126:130, :])
            nc.gpsimd.dma_start(out=x63[:, 1, :, :], in_=xc[128:256, 126:130, :])
            for ch in range(2):
                acc = fpool.tile([128, h], f32, tag="acc63")
                tmp = fpool.tile([128, h], f32, tag="tmp63")
                for t in range(16):
                    kh, kw = t // 4, t % 4
                    src = x63[:, ch, kh, kw:kw + 2 * (w - 1) + 1:2]
                    if t == 0:
                        nc.vector.tensor_scalar_mul(acc, src, wdev_sb[:, ch, t:t + 1])
                    else:
                        nc.vector.tensor_scalar_mul(tmp, src, wdev_sb[:, ch, t:t + 1])
                        nc.vector.tensor_add(acc, acc, tmp)
                nc.gpsimd.dma_start(
                    out=stage[ch * 128:(ch + 1) * 128, 63 * w:64 * w], in_=acc)

            a_t = p_at.tile([128, 2, Ca, h], bf16)   # [W-half, half, ca, i]
            for g in range(C // CG):                 # 32 groups of 8 channels
                c0 = g * CG
                xt = cpool.tile([128, CG, 2, W], bf16, tag="xt")
                nc.gpsimd.dma_start(
                    out=xt[:, :, 0, :],
                    in_=xc[c0:c0 + CG, 0:128, :].rearrange("c y x -> y c x"))
                nc.gpsimd.dma_start(
                    out=xt[:, :, 1, :],
                    in_=xc[c0:c0 + CG, 128:256, :].rearrange("c y x -> y c x"))
                if g % 2 == 0:
                    wt = wpool.tile([128, 16, 4, 64], bf16, tag="wt")
                    for wh in range(2):
                        nc.gpsimd.dma_start(
                            out=wt[:, 8 * wh:8 * wh + 8].rearrange(
                                "r c k m -> r (c k m)"),
                            in_=wconv[:, c0 + 8 * wh:c0 + 8 * wh + 8].rearrange(
                                "r c k m -> r (c k m)"))
                fU = fpool.tile([63, CG, w], bf16, tag="fU")
                fL = fpool.tile([63, CG, w], bf16, tag="fL")
                for cpair in range(CG // 2):   # 2 channels per PSUM (col-tiled)
                    ci0 = cpair * 2
                    ps = ps_cv.tile([128, 2, 127], f32, tag="cvps")
                    for kw in range(4):
                        for d in range(2):
                            nc.tensor.matmul(
                                ps[64 * d:64 * d + 64],
                                wt[:, (c0 + ci0 + d) % 16, kw, :],
                                xt[:, ci0 + d, :, kw:kw + 2 * (w - 1) + 1:2],
                                start=(kw == 0), stop=(kw == 3))
                    for d in range(2):
                        b = 64 * d
                        if (cpair + d) % 2 == 0:
                            nc.scalar.copy(fU[:, ci0 + d, :], ps[b:b + 63, 0, :])
                            nc.vector.tensor_copy(fL[:, ci0 + d, :], ps[b:b + 63, 1, :])
                        else:
                            nc.vector.tensor_copy(fU[:, ci0 + d, :], ps[b:b + 63, 0, :])
                            nc.scalar.copy(fL[:, ci0 + d, :], ps[b:b + 63, 1, :])
                nc.gpsimd.dma_start(
                    out=stage[c0:c0 + CG, 0:63 * w].rearrange(
                        "c (i j) -> i c j", i=63), in_=fU)
                nc.gpsimd.dma_start(
                    out=stage[c0:c0 + CG, 64 * w:].rearrange(
                        "c (i j) -> i c j", i=63), in_=fL)

                # interleave 2 attention-map channels per group
                if g % 4 == 0:
                    amt = apool.tile([128, 8, 2, W], bf16, tag="amt")
                    ca0 = (g // 4) * 8
                    nc.gpsimd.dma_start(
                        out=amt[:, :, 0, :],
                        in_=amc[ca0:ca0 + 8, 0:128, :].rearrange("c y x -> y c x"))
                    nc.gpsimd.dma_start(
                        out=amt[:, :, 1, :],
                        in_=amc[ca0:ca0 + 8, 128:256, :].rearrange("c y x -> y c x"))
                for dca in range(2):
                    ca = g * 2 + dca
                    ai = ca % 8
                    psr = ps_a.tile([h, W], f32, tag="aps")
                    nc.tensor.matmul(psr, rdown_sb[:, 0, :], amt[:, ai, 0, :],
                                     start=True, stop=False)
                    nc.tensor.matmul(psr, rdown_sb[:, 1, :], amt[:, ai, 1, :],
                                     start=False, stop=True)
                    mid = apool.tile([h, W], bf16, tag="mid")
                    nc.scalar.copy(mid, psr)
                    for wh in range(2):
                        pst = ps_a.tile([128, h], bf16, tag="apst")
                        nc.tensor.transpose(pst, mid[:, wh * 128:(wh + 1) * 128],
                                            ident[0:h, 0:h])
                        nc.vector.tensor_copy(a_t[:, wh, ca, :], pst)

        # a-map col-resize -> a_dram (bounced; reloaded at phase 5)
        with tc.tile_pool(name="ps_a2", bufs=3, space="PSUM") as ps_a2, \
             tc.tile_pool(name="a2tmp", bufs=2) as a2tmp:
            for i0 in range(0, h, 8):
                ib = min(8, h - i0)
                at8 = a2tmp.tile([Ca, 8, h], bf16, tag="at8")
                for di in range(ib):
                    i = i0 + di
                    psc = ps_a2.tile([Ca, h], f32, tag="aps2")
                    nc.tensor.matmul(psc, a_t[:, 0, :, i], cdown_sb[:, 0, :],
                                     start=True, stop=False)
                    nc.tensor.matmul(psc, a_t[:, 1, :, i], cdown_sb[:, 1, :],
                                     start=False, stop=True)
                    nc.scalar.copy(at8[:, di, :], psc)
                nc.gpsimd.dma_start(
                    out=a_dram[:, i0 * w:(i0 + ib) * w],
                    in_=at8[:, 0:ib, :].rearrange("c i j -> c (i j)"))
        cm_at.__exit__(None, None, None)

        # ================ Phase 2+3: q/k and v projections ================
        cm_vw = tc.tile_pool(name="p_vw", bufs=1, side="right")
        p_vw = cm_vw.__enter__()
        v_w = p_vw.tile([h, NI, C], bf16)  # [m, (i, c)] -- stays resident
        cm_xd = tc.tile_pool(name="p_xd", bufs=1, side="right")
        p_xd = cm_xd.__enter__()
        xd0 = p_xd.tile([128, PIX], bf16)
        xd1 = p_xd.tile([128, PIX], bf16)
        for lo in range(0, PIX, 2048):
            hi = min(lo + 2048, PIX)
            nc.gpsimd.dma_start(out=xd0[:, lo:hi], in_=stage[0:128, lo:hi])
            nc.gpsimd.dma_start(out=xd1[:, lo:hi], in_=stage[128:256, lo:hi])

        cm_qk = tc.tile_pool(name="p_qk", bufs=1)
        p_qk = cm_qk.__enter__()
        q_sb = p_qk.tile([32, NI * w], bf16)
        k_sb = p_qk.tile([32, PIX], bf16)
        with tc.tile_pool(name="ps_qk", bufs=3, space="PSUM") as ps_qk:
            CH = 508
            for k in range((PIX + CH - 1) // CH):
                lo = k * CH
                hi = min(lo + CH, PIX)
                ps = ps_qk.tile([64, CH], f32, tag="qkps")
                nc.tensor.matmul(ps[:, 0:hi - lo], wqkt_sb[:, 0, :], xd0[:, lo:hi],
                                 start=True, stop=False)
                nc.tensor.matmul(ps[:, 0:hi - lo], wqkt_sb[:, 1, :], xd1[:, lo:hi],
                                 start=False, stop=True)
                if lo < NI * w:
                    qhi = min(hi, NI * w)
                    nc.scalar.activation(out=q_sb[:, lo:qhi], in_=ps[0:32, 0:qhi - lo],
                                         func=Ident, bias=bqk_sb[0:32], scale=1.0)
                nc.scalar.activation(out=k_sb[:, lo:hi], in_=ps[32:64, 0:hi - lo],
                                     func=Ident, bias=bqk_sb[32:64], scale=1.0)

        with tc.tile_pool(name="ps_v", bufs=3, space="PSUM") as ps_v, \
             tc.tile_pool(name="vttmp", bufs=4) as vttmp:
            for i in range(NI):
                ps = ps_v.tile([h, C], f32, tag="vps")
                nc.tensor.matmul(ps, xd0[:, i * w:(i + 1) * w], wvt_sb[:, 0, :],
                                 start=True, stop=False)
                nc.tensor.matmul(ps, xd1[:, i * w:(i + 1) * w], wvt_sb[:, 1, :],
                                 start=False, stop=True)
                nc.vector.tensor_add(v_w[:, i, :], ps, bv_bc[0:h, :])
            for j0 in range(0, h, 16):
                jb = min(16, h - j0)
                vtt = vttmp.tile([h, 16, C], bf16, tag="vtt")
                for dj in range(jb):
                    j = j0 + dj
                    ps = ps_v.tile([h, C], f32, tag="vps")
                    nc.tensor.matmul(ps, xd0[:, j:j + (h - 1) * w + 1:w],
                                     wvt_sb[:, 0, :], start=True, stop=False)
                    nc.tensor.matmul(ps, xd1[:, j:j + (h - 1) * w + 1:w],
                                     wvt_sb[:, 1, :], start=False, stop=True)
                    nc.vector.tensor_add(vtt[:, dj, :], ps, bv_bc[0:h, :])
                for wh in range(0, jb, 8):
                    we = min(wh + 8, jb)
                    nc.gpsimd.dma_start(
                        out=vt_dram[:, j0 + wh:j0 + we, :].rearrange(
                            "l j c -> l (j c)"),
                        in_=vtt[:, wh:we, :].rearrange("l j c -> l (j c)"))
        cm_xd.__exit__(None, None, None)

        cm_a = tc.tile_pool(name="p_a", bufs=1)
        p_a = cm_a.__enter__()
        a_sb = p_a.tile([Ca, PIX], bf16)
        nc.sync.dma_start(out=a_sb, in_=a_dram)

        # ================= Phase 5: energies + first softmax ==============
        cm_e = tc.tile_pool(name="p_e", bufs=1, side="right")
        p_e = cm_e.__enter__()
        peH2 = p_e.tile([128, 16, 4, h], bf16)  # [(jhalf,i), blk, dj, l]
        peW = p_e.tile([h, NI, h], bf16)   # [j, (i, m)]
        zeH2 = p_e.tile([128, 64], f32)
        zaH2 = p_e.tile([128, 64], f32)
        zeW = p_e.tile([h, NI], f32)
        zaW = p_e.tile([h, NI], f32)
        with tc.tile_pool(name="espool", bufs=3) as espool, \
             tc.tile_pool(name="ps_e", bufs=3, space="PSUM") as ps_e:
            Red = mybir.AluOpType.add
            AxX = mybir.AxisListType.X
            for blk in range(16):
                jj0 = blk * 4
                pe = ps_e.tile([128, 4, 128], f32, tag="pe")
                pa = ps_e.tile([128, 4, 128], f32, tag="pa")
                for jhalf in range(2):
                    b = 64 * jhalf
                    for dj in range(4 if jhalf == 0 else min(4, 63 - jj0)):
                        jj = jhalf * 64 + jj0 + dj
                        nc.tensor.matmul(pe[b:b + 64, dj, 0:h],
                                         q_sb[:, jj:jj + 63 * w + 1:w],
                                         k_sb[:, jj:jj + (h - 1) * w + 1:w],
                                         start=True, stop=True)
                        nc.tensor.matmul(pa[b:b + 64, dj, 0:h],
                                         a_sb[:, jj:jj + 63 * w + 1:w],
                                         a_sb[:, jj:jj + (h - 1) * w + 1:w],
                                         start=True, stop=True)
                nc.vector.tensor_add(pe[:, :, 0:h], pe[:, :, 0:h], dmask8)
                nc.vector.tensor_add(pa[:, :, 0:h], pa[:, :, 0:h], dmask8)
                ee = espool.tile([128, 4, h], bf16, tag="ee")
                nc.scalar.activation(out=ee, in_=pe[:, :, 0:h],
                                     func=Exp, bias=eshE_sb)
                ea = espool.tile([128, 4, h], bf16, tag="ea")
                nc.scalar.activation(out=ea, in_=pa[:, :, 0:h],
                                     func=Exp, bias=eshA_sb)
                nc.vector.tensor_reduce(zeH2[:, jj0:jj0 + 4], ee, axis=AxX, op=Red)
                nc.vector.tensor_reduce(zaH2[:, jj0:jj0 + 4], ea, axis=AxX, op=Red)
                nc.vector.tensor_mul(peH2[:, blk, :, :], ee, ea)
        with tc.tile_pool(name="espool2", bufs=3) as espool, \
             tc.tile_pool(name="ps_e2", bufs=3, space="PSUM") as ps_e:
            Red = mybir.AluOpType.add
            AxX = mybir.AxisListType.X
            for i0 in range(0, NI, 4):
                pe = ps_e.tile([h, 4, 128], f32, tag="pew")
                pa = ps_e.tile([h, 4, 128], f32, tag="paw")
                for di in range(4):
                    ii = i0 + di
                    nc.tensor.matmul(pe[:, di, 0:h], q_sb[:, ii * w:(ii + 1) * w],
                                     k_sb[:, ii * w:(ii + 1) * w],
                                     start=True, stop=True)
                    nc.tensor.matmul(pa[:, di, 0:h], a_sb[:, ii * w:(ii + 1) * w],
                                     a_sb[:, ii * w:(ii + 1) * w],
                                     start=True, stop=True)
                ee = espool.tile([h, 4, h], bf16, tag="eew")
                nc.scalar.activation(out=ee, in_=pe[:, :, 0:h], func=Exp, bias=eshE_sb[0:h])
                ea = espool.tile([h, 4, h], bf16, tag="eaw")
                nc.scalar.activation(out=ea, in_=pa[:, :, 0:h], func=Exp, bias=eshA_sb[0:h])
                nc.vector.tensor_reduce(zeW[:, i0:i0 + 4], ee, axis=AxX, op=Red)
                nc.vector.tensor_reduce(zaW[:, i0:i0 + 4], ea, axis=AxX, op=Red)
                nc.vector.tensor_mul(peW[:, i0:i0 + 4, :], ee, ea)
        cm_a.__exit__(None, None, None)
        cm_qk.__exit__(None, None, None)

        # v (column layout) reload -- overlaps 6a (space freed by q/k/a_sb)
        cm_vt = tc.tile_pool(name="p_vt", bufs=1)
        p_vt = cm_vt.__enter__()
        v_t = p_vt.tile([h, h, C], bf16)   # [l, (j, c)]
        for j0 in range(0, h, 16):
            jb = min(16, h - j0)
            nc.gpsimd.dma_start(
                out=v_t[:, j0:j0 + jb, :].rearrange("l j c -> l (j c)"),
                in_=vt_dram[:, j0:j0 + jb, :].rearrange("l j c -> l (j c)"))

        # normalizer merge (stacked): rs = 1/((zeH+zeW^T)*(zaH+zaW^T))
        rs_H2 = consts.tile([128, 64], f32)
        rs_W = consts.tile([h, NI], f32)
        zsH2 = consts.tile([128, 64], f32)
        zsW = consts.tile([h, NI], f32)
        zs_tot = consts.tile([h, NI], f32)
        with tc.tile_pool(name="ps_s", bufs=2, space="PSUM") as ps_s, \
             tc.tile_pool(name="stmp", bufs=1) as stmp:
            zeW16 = stmp.tile([h, NI], bf16)
            zaW16 = stmp.tile([h, NI], bf16)
            nc.vector.tensor_copy(zeW16, zeW)
            nc.vector.tensor_copy(zaW16, zaW)
            pz1 = ps_s.tile([128, NI], bf16, tag="pz")
            nc.tensor.transpose(pz1[0:64, :], zeW16[0:64, :], ident[0:64, 0:64])
            nc.tensor.transpose(pz1[64:128, 0:63], zeW16[64:127, :],
                                ident[64:127, 64:127])
            ze_tot = stmp.tile([128, 64], f32)
            nc.vector.tensor_add(ze_tot, zeH2, pz1[:, 0:64])
            pz2 = ps_s.tile([128, NI], bf16, tag="pz")
            nc.tensor.transpose(pz2[0:64, :], zaW16[0:64, :], ident[0:64, 0:64])
            nc.tensor.transpose(pz2[64:128, 0:63], zaW16[64:127, :],
                                ident[64:127, 64:127])
            za_tot = stmp.tile([128, 64], f32)
            nc.vector.tensor_add(za_tot, zaH2, pz2[:, 0:64])
            nc.vector.tensor_mul(rs_H2, ze_tot, za_tot)
            nc.vector.reciprocal(rs_H2, rs_H2)
            rs16 = stmp.tile([128, 64], bf16)
            nc.vector.tensor_copy(rs16, rs_H2)
            pz3 = ps_s.tile([128, NI], bf16, tag="pzw")
            nc.tensor.transpose(pz3[0:64, :], rs16[0:64, :], ident[0:64, 0:64])
            nc.tensor.transpose(pz3[64:128, :], rs16[64:128, :],
                                ident[64:128, 64:128])
            nc.vector.tensor_copy(rs_W, pz3[0:h, :])

        # ====== Phase 6a: second softmax -> transposed attention weights ===
        cm_es = tc.tile_pool(name="p_es", bufs=1)
        p_es = cm_es.__enter__()
        esT_H2 = p_es.tile([h, 16, 4, 128], bf16)  # [l, blk, dj, (jhalf,i)]
        esT_W = p_es.tile([h, NI, h], bf16)   # [m, (i, j)]
        with tc.tile_pool(name="spool", bufs=2) as spool, \
             tc.tile_pool(name="ps_f", bufs=3, space="PSUM") as ps_f:
            Red = mybir.AluOpType.add
            AxX = mybir.AxisListType.X
            for blk in range(16):
                jj0 = blk * 4
                p1 = spool.tile([128, 4, h], bf16, tag="p1")
                rs_bc = rs_H2[:, jj0:jj0 + 4].to_broadcast((128, 4, h))
                nc.vector.tensor_tensor(out=p1, in0=peH2[:, blk, :, :],
                                        in1=rs_bc, op=mybir.AluOpType.mult)
                es = spool.tile([128, 4, h], bf16, tag="es")
                nc.scalar.activation(out=es, in_=p1, func=Exp)
                nc.vector.tensor_reduce(zsH2[:, jj0:jj0 + 4], es, axis=AxX, op=Red)
                pt = ps_f.tile([h, 4, 128], bf16, tag="pt")
                for dj in range(4):
                    nc.tensor.transpose(pt[:, dj, :], es[:, dj, :], ident)
                nc.vector.tensor_copy(esT_H2[:, blk, :, :], pt)
            for i0 in range(0, NI, 4):
                p1 = spool.tile([h, 4, h], bf16, tag="p1w")
                rs_bc = rs_W[:, i0:i0 + 4].to_broadcast((h, 4, h))
                nc.vector.tensor_tensor(out=p1, in0=peW[:, i0:i0 + 4, :],
                                        in1=rs_bc, op=mybir.AluOpType.mult)
                es = spool.tile([h, 4, h], bf16, tag="esw")
                nc.scalar.activation(out=es, in_=p1, func=Exp)
                nc.vector.tensor_reduce(zsW[:, i0:i0 + 4], es, axis=AxX, op=Red)
                pt = ps_f.tile([h, 8, 128], bf16, tag="ptw")
                for di in range(4):
                    nc.tensor.transpose(pt[:, di, 0:h], es[:, di, :], ident[0:h, 0:h])
                nc.vector.tensor_copy(esT_W[:, i0:i0 + 4, :], pt[:, 0:4, 0:h])
        cm_e.__exit__(None, None, None)

        cm_o = tc.tile_pool(name="p_o", bufs=1)
        p_o = cm_o.__enter__()
        Tbuf = p_o.tile([h, NI, C], bf16)     # [j, (i, c)]

        # zs_tot = 1/(zsH^T + zsW)  [h, NI]
        with tc.tile_pool(name="ps_m", bufs=2, space="PSUM") as ps_m, \
             tc.tile_pool(name="mtmp", bufs=1) as mtmp:
            zsH16 = mtmp.tile([128, 64], bf16)
            nc.vector.tensor_copy(zsH16, zsH2)
            pzs = ps_m.tile([128, NI], bf16, tag="pzs")
            nc.tensor.transpose(pzs[0:64, :], zsH16[0:64, :], ident[0:64, 0:64])
            nc.tensor.transpose(pzs[64:128, :], zsH16[64:128, :],
                                ident[64:128, 64:128])
            nc.vector.tensor_add(zs_tot, zsW, pzs[0:h, :])
            nc.vector.reciprocal(zs_tot, zs_tot)

        # ====== Phase 6b: attention apply (outW then outH) -> Tbuf ========
        with tc.tile_pool(name="ps_f3", bufs=3, space="PSUM") as ps_f3:
            for i0 in range(0, NI, 2):
                po = ps_f3.tile([h, 2, C], f32, tag="pow")
                for di in range(2):
                    nc.tensor.matmul(po[:, di, :], esT_W[:, i0 + di, :],
                                     v_w[:, i0 + di, :], start=True, stop=True)
                nc.vector.tensor_copy(Tbuf[:, i0:i0 + 2, :], po)
        cm_vw.__exit__(None, None, None)
        with tc.tile_pool(name="p_oh", bufs=2) as p_oh, \
             tc.tile_pool(name="ps_f2", bufs=3, space="PSUM") as ps_f2:
            for ch in range(2):
                outHh = p_oh.tile([NI, h, 128], bf16, tag="outHh")
                for jb in range(32):
                    jhalf, blk = jb // 16, jb % 16
                    j0 = jhalf * 64 + blk * 4
                    jn = min(4, h - j0)
                    po = ps_f2.tile([NI, 4, 128], f32, tag="po")
                    for dj in range(jn):
                        nc.tensor.matmul(
                            po[:, dj, :],
                            esT_H2[:, blk, dj, 64 * jhalf:64 * jhalf + 64],
                            v_t[:, j0 + dj, ch * 128:(ch + 1) * 128],
                            start=True, stop=True)
                    if (j0 // 4) % 2 == 0:
                        nc.vector.tensor_copy(outHh[:, j0:j0 + jn, :], po[:, 0:jn, :])
                    else:
                        nc.scalar.copy(outHh[:, j0:j0 + jn, :], po[:, 0:jn, :])
                for cb0 in range(0, 128, 4):
                    ptr = ps_f2.tile([h, 8, 128], bf16, tag="ptr")
                    for db in range(4):
                        nc.tensor.transpose(ptr[:, db, 0:NI], outHh[:, :, cb0 + db],
                                            ident[0:NI, 0:NI])
                    dst = Tbuf[:, :, ch * 128 + cb0:ch * 128 + cb0 + 4]
                    src = ptr[:, 0:4, 0:NI].rearrange("p c i -> p i c")
                    nc.vector.tensor_tensor(out=dst, in0=dst, in1=src, op=Add)

        # T *= zs_tot
        for i in range(NI):
            nc.vector.tensor_scalar_mul(Tbuf[:, i, :], Tbuf[:, i, :],
                                        zs_tot[:, i:i + 1])

        # ====== Phase 7: upsample + residual, pipelined in 64-ch blocks ====
        Rg = 8    # residual channels per DMA group
        CQ = 64   # channels per pipeline block
        with tc.tile_pool(name="p_u", bufs=1) as p_u, \
             tc.tile_pool(name="p_r", bufs=2) as p_r, \
             tc.tile_pool(name="ps_u", bufs=3, space="PSUM") as ps_u, \
             tc.tile_pool(name="ps_r", bufs=3, space="PSUM") as ps_r:
            for cq in range(C // CQ):                # 4 blocks of 64 channels
                cqb = cq * CQ
                U2 = p_u.tile([NI, CQ, W], bf16, tag="U2")   # [i, c, xo]
                for c in range(CQ):
                    psc = ps_u.tile([NI, W], f32, tag="ups")
                    nc.tensor.matmul(psc, Tbuf[:, :, cqb + c], cup_sb,
                                     start=True, stop=True)
                    if c % 2 == 0:
                        nc.scalar.copy(U2[:, c, :], psc)
                    else:
                        nc.vector.tensor_copy(U2[:, c, :], psc)
                U2f = U2.rearrange("i c xo -> i (c xo)")
                for g8 in range(CQ // Rg):           # 8 groups of 8 channels
                    c0 = cqb + g8 * Rg
                    xr8 = p_r.tile([NO, Rg, W], bf16, tag="xr8")
                    nc.gpsimd.dma_start(
                        out=xr8,
                        in_=xc[c0:c0 + Rg, 0:NO, :].rearrange("c y xo -> y c xo"))
                    o8 = p_r.tile([NO, Rg, W], f32, tag="o8")
                    for k in range(Rg // 2):         # 2 channels per matmul
                        kk = g8 * (Rg // 2) + k
                        ps = ps_r.tile([NO, 512], f32, tag="rps")
                        nc.tensor.matmul(ps, rupg_sb,
                                         U2f[:, kk * 512:(kk + 1) * 512],
                                         start=True, stop=True)
                        nc.vector.tensor_tensor(
                            out=o8[:, 2 * k:2 * k + 2, :].rearrange("p c xo -> p (c xo)"),
                            in0=ps,
                            in1=xr8[:, 2 * k:2 * k + 2, :].rearrange("p c xo -> p (c xo)"),
                            op=Add)
                    nc.gpsimd.dma_start(
                        out=out_t[c0:c0 + Rg, :, :].rearrange("c y xo -> y c xo"),
                        in_=o8)
        cm_o.__exit__(None, None, None)
        cm_es.__exit__(None, None, None)
        cm_vt.__exit__(None, None, None)
        cm_dram.__exit__(None, None, None)
        cm_consts.__exit__(None, None, None)

    return nc


_NC_CACHE = {}


def _get_nc():
    if "nc" not in _NC_CACHE:
        _install_hookfix()
        import concourse.mybir as mybir
        nc = _build_kernel()
        _split_multi_waits(nc.m, mybir)
        _NC_CACHE["nc"] = nc
    return _NC_CACHE["nc"]


def _host_inputs(x, attention_map, w_down, wq, bq, wk, bk, wv, bv, gamma):
    import ml_dtypes
    bfl = ml_dtypes.bfloat16
    x = np.ascontiguousarray(x, _f32)
    attention_map = np.ascontiguousarray(attention_map, _f32)
    w_down = np.asarray(w_down, _f32)
    gamma_v = float(np.asarray(gamma).reshape(-1)[0])

    wqkt = np.concatenate([np.asarray(wq, _f32).T, np.asarray(wk, _f32).T], axis=1)
    bqk = np.concatenate([np.asarray(bq, _f32), np.asarray(bk, _f32)])[:, None]
    wvt = np.asarray(wv, _f32).T.copy()
    bv_ = np.asarray(bv, _f32)[None, :]
    rdown = _resize_mat(H, h)
    cdown = _resize_mat(W, h)
    cup = _resize_mat(h, W)
    rupg = np.ascontiguousarray(_resize_mat(h, H)[:NI, :NO] * gamma_v)
    dmask = np.zeros((NI, h), _f32)
    dmask[np.arange(NI), np.arange(NI)] = -30000.0

    wconv_n = _conv_weights(w_down).astype(bfl)
    wconv_f = _conv_weights(w_down[:, :, ::-1, :]).astype(bfl)
    wdev_n = np.ascontiguousarray(w_down[:, 0].reshape(C, 16))
    wdev_f = np.ascontiguousarray(w_down[:, 0, ::-1, :].reshape(C, 16))

    shared = dict(wqkt=wqkt, bqk=bqk, wvt=wvt, bv=bv_, rdown=rdown,
                  cdown=cdown, cup=cup, rupg=rupg, dmask=dmask)
    in_maps = []
    x16 = x.astype(bfl)
    am16 = attention_map.astype(bfl)
    for core in range(8):
        b, half = core // 2, core % 2
        if half == 0:
            m = dict(xc=x16[b], amc=am16[b], wconv=wconv_n, wdev=wdev_n)
        else:
            m = dict(xc=np.ascontiguousarray(x16[b, :, ::-1, :]),
                     amc=np.ascontiguousarray(am16[b, :, ::-1, :]),
                     wconv=wconv_f, wdev=wdev_f)
        m.update(shared)
        in_maps.append(m)
    return in_maps


def kernel(x, attention_map, w_down, wq, bq, wk, bk, wv, bv, gamma):
    _install_hookfix()
    from concourse import bass_utils

    nc = _get_nc()
    in_maps = _host_inputs(x, attention_map, w_down, wq, bq, wk, bk, wv, bv, gamma)
    res = bass_utils.run_bass_kernel_spmd(nc, in_maps, core_ids=list(range(8)))
    out = np.empty((B, C, H, W), _f32)
    for core in range(8):
        b, half = core // 2, core % 2
        o = res.results[core]["out"]
        if half == 0:
            out[b, :, 0:NO, :] = o
        else:
            out[b, :, NO:H, :] = o[:, ::-1, :]
    return out
